# revision 20
# baseline (speedup 1.0000x reference)
"""Correlation-layer cosine-similarity kernel for Trainium2 (8 NeuronCores).

Problem: x1, x2: [B=4, C=256, H=128, W=256] fp32.
out[b, d, h, w] = cos-sim over C of (x1[b,:,h,w], x2_padded[b,:,h,w+d]), d in 0..40.

Sharding: core i handles batch b = i//2 and H-half hh = i%2 (64 rows).

v2 design (per core):
- Inputs converted to bf16 on the host (halves input HBM traffic; numerically
  equivalent to the DMA-cast the baseline already did).
- Per h row: Gram cover G[w1, w1..w1+40] via 4 bf16 matmuls of N=168
  (2 c-chunks x 2 w-blocks) into PSUM.
- Norms are folded into the operands instead of staged/skew-read:
  * per h one [1, 512] ones-matmul row = (n1^2 | n2^2); Sqrt activation
    drains it to an SBUF [8, 512] stack; reciprocal_approx_fast + bf16 copy.
  * n2: rsqrt row is broadcast to 128 partitions with a K=1 ones matmul and
    multiplied into x2 before the Gram (columns pre-normalized).
  * n1: rsqrt row PE-transposed to per-partition layout [128, (mc, h)] and
    applied as the activation-scale during the PSUM->SBUF cover copy.
- Diagonal (d) extraction needs a DRAM round-trip (per-partition offsets are
  inexpressible in SBUF APs), but covers of HG=8 h-rows are interleaved
  [p, mc, col, h] in bf16 so the skewed read-back has 656B contiguous chunks
  and the cover write is one 5.4KB/partition DMA per group.
- Band [128w, 2mc, 41d, 8h] is PE-transposed per h to [82, 128] and written
  out f32 as two [41, 8, 128] DMAs per group.
"""

import numpy as np

B, C, H, W = 4, 256, 128, 256
D = 41          # displacements 0..40
HC = 64         # H rows per core
PAD = 40
W2 = W + PAD    # 296
COVER = 168     # cols of G covering the diag band per 128-row block
HG = 8          # h rows per group (stage/interleave unit)
NG = HC // HG   # 8 groups

_cache = {}


def _build_nc():
    import concourse.bass as bass
    import concourse.tile as tile
    from concourse import bacc, mybir
    from concourse.masks import make_identity

    f32 = mybir.dt.float32
    bf16 = mybir.dt.bfloat16
    Act = mybir.ActivationFunctionType

    nc = bacc.Bacc(trn_type="TRN2")
    x1s = nc.dram_tensor("x1s", [C, HC, W], bf16, kind="ExternalInput")
    x2s = nc.dram_tensor("x2s", [C, HC, W], bf16, kind="ExternalInput")
    outs = nc.dram_tensor("outs", [D, HC, W], f32, kind="ExternalOutput")
    # per-group DRAM scratch, interleaved [p, mc, col, h_sub]
    gds = [
        nc.dram_tensor(f"gd{g}", [128, 2, COVER, HG], bf16, kind="Internal")
        for g in range(NG)
    ]

    with tile.TileContext(nc) as tc:
        with (
            tc.tile_pool(name="const", bufs=1) as constp,
            tc.tile_pool(name="io", bufs=2) as io,
            tc.tile_pool(name="sqp", bufs=2) as sqp,
            tc.tile_pool(name="x2np", bufs=3) as x2np,
            tc.tile_pool(name="rowp", bufs=2) as rowp,
            tc.tile_pool(name="gsbp", bufs=2) as gsbp,
            tc.tile_pool(name="bandp", bufs=2) as bandp,
            tc.tile_pool(name="outp", bufs=2) as outp,
            tc.tile_pool(name="gp", bufs=2, space="PSUM") as gp,
            tc.tile_pool(name="rp", bufs=2, space="PSUM") as rp,
            tc.tile_pool(name="nrowp", bufs=2, space="PSUM") as nrowp,
            tc.tile_pool(name="tpp", bufs=2, space="PSUM") as tpp,
        ):
            ones = constp.tile([128, 1], bf16)
            nc.vector.memset(ones, 1.0)
            onesrow = constp.tile([1, 128], bf16)
            nc.vector.memset(onesrow, 1.0)
            ident = constp.tile([128, 128], bf16)
            make_identity(nc, ident)
            identf = constp.tile([128, 128], f32)
            make_identity(nc, identf)
            epsb = constp.tile([128, 1], f32)
            nc.vector.memset(epsb, 1e-12)

            for g in range(NG):
                h0 = g * HG
                x1blk = io.tile([128, 2, HG, W], bf16, tag="x1blk")
                x2blk = io.tile([128, 2, HG, W], bf16, tag="x2blk")
                for kc in range(2):
                    nc.sync.dma_start(
                        out=x1blk[:, kc],
                        in_=x1s[kc * 128:(kc + 1) * 128, h0:h0 + HG, :])
                    nc.sync.dma_start(
                        out=x2blk[:, kc],
                        in_=x2s[kc * 128:(kc + 1) * 128, h0:h0 + HG, :])

                # squares packed [128, kc, h, (sq1|sq2)]
                sq12 = sqp.tile([128, 2, HG, 2 * W], bf16, tag="sq12")
                nc.vector.tensor_mul(sq12[:, :, :, 0:W], x1blk, x1blk)
                nc.vector.tensor_mul(sq12[:, :, :, W:2 * W], x2blk, x2blk)

                # norms: per h one [1, 512] psum row (n1sq | n2sq); engines
                # may write SBUF only at partition 0, so drain rows into
                # partition-0 [1, 2048] rows (1-lane copies), then restack n1
                # to [8, 256] with one SBUF->SBUF DMA for the PE transpose.
                r1sq = rowp.tile([1, HG * W], f32, tag="r1sq")
                r2sq = rowp.tile([1, HG * W], bf16, tag="r2sq")
                for j in range(HG):
                    nrow = nrowp.tile([1, 2 * W], f32, tag="nrow")
                    for kc in range(2):
                        nc.tensor.matmul(nrow, ones, sq12[:, kc, j, :],
                                         start=(kc == 0), stop=(kc == 1))
                    nc.scalar.copy(r1sq[:, j * W:(j + 1) * W],
                                   nrow[:, 0:W])
                    nc.vector.tensor_copy(r2sq[:, j * W:(j + 1) * W],
                                          nrow[:, W:2 * W])
                n1stk = rowp.tile([HG, W], f32, tag="n1stk")
                nc.sync.dma_start(
                    out=n1stk, in_=r1sq.rearrange("p (h w) -> p h w", h=HG))

                # n1: transpose n1stk -> [128, (mc, h)], rsqrt on wide layout
                n1ss = rowp.tile([128, 2, HG], f32, tag="n1ss")
                for mc in range(2):
                    n1tp = tpp.tile([128, HG], f32, tag="tp")
                    nc.tensor.transpose(
                        n1tp, n1stk[:, mc * 128:(mc + 1) * 128],
                        identf[0:HG, 0:HG])
                    nc.scalar.activation(out=n1ss[:, mc, :], in_=n1tp,
                                         func=Act.Sqrt, bias=epsb)
                n1t = rowp.tile([128, 2, HG], f32, tag="n1t")
                nc.vector.reciprocal_approx_fast(
                    out=n1t.rearrange("p a b -> p (a b)"),
                    in_=n1ss.rearrange("p a b -> p (a b)"))

                gsb = gsbp.tile([128, 2, COVER, HG], bf16, tag="gsb")
                for j in range(HG):
                    # broadcast raw n2sq row j (K=1 ones matmul), rsqrt wide
                    r2bc = rp.tile([128, W], f32, tag="r2bc")
                    nc.tensor.matmul(r2bc, onesrow,
                                     r2sq[:, j * W:(j + 1) * W],
                                     start=True, stop=True)
                    r2s = x2np.tile([128, W], f32, tag="r2s")
                    nc.scalar.activation(out=r2s, in_=r2bc,
                                         func=Act.Sqrt, bias=epsb)
                    r2r = x2np.tile([128, W], f32, tag="r2r")
                    nc.vector.reciprocal_approx_fast(out=r2r, in_=r2s)
                    x2n = x2np.tile([128, 2, W2], bf16, tag="x2n")
                    nc.vector.memset(x2n[:, :, W:W2], 0.0)
                    nc.vector.tensor_mul(
                        x2n[:, :, 0:W], x2blk[:, :, j, :],
                        r2r.unsqueeze(1).broadcast_to([128, 2, W]))

                    gps = gp.tile([128, 2, COVER], f32, tag="gg")
                    for kc in range(2):
                        nc.tensor.matmul(gps[:, 0, :],
                                         x1blk[:, kc, j, 0:128],
                                         x2n[:, kc, 0:COVER],
                                         start=(kc == 0), stop=(kc == 1))
                    for kc in range(2):
                        nc.tensor.matmul(gps[:, 1, :],
                                         x1blk[:, kc, j, 128:256],
                                         x2n[:, kc, 128:W2],
                                         start=(kc == 0), stop=(kc == 1))
                    # n1 scale folded into PSUM->SBUF cover copy
                    nc.scalar.activation(
                        out=gsb[:, 0, :, j], in_=gps[:, 0, :],
                        func=Act.Copy, scale=n1t[:, 0, j:j + 1])
                    nc.vector.tensor_scalar_mul(
                        out=gsb[:, 1, :, j], in0=gps[:, 1, :],
                        scalar1=n1t[:, 1, j:j + 1])

                nc.gpsimd.dma_start(out=gds[g][:], in_=gsb)

                # skewed diag read-back: band[p, mc, d, h] = cover[p, mc, p+d, h]
                band = bandp.tile([128, 2, D, HG], bf16, tag="band")
                src = bass.AP(tensor=gds[g], offset=0,
                              ap=[[(2 * COVER + 1) * HG, 128],
                                  [COVER * HG, 2], [HG, D], [1, HG]])
                nc.gpsimd.dma_start(out=band, in_=src)

                out_sb = outp.tile([D, HG, W], f32, tag="out_sb")
                for j in range(HG):
                    tp2 = tpp.tile([D, W], bf16, tag="tp")
                    for mc in range(2):
                        nc.tensor.matmul(
                            tp2[:, mc * 128:(mc + 1) * 128],
                            band[:, mc, :, j], ident,
                            start=True, stop=True, is_transpose=True)
                    nc.scalar.copy(out_sb[:, j, :], tp2)
                dst = bass.AP(tensor=outs, offset=h0 * W,
                              ap=[[HC * W, D], [1, HG * W]])
                nc.sync.dma_start(out=dst,
                                  in_=out_sb.rearrange("d h w -> d (h w)"))

    nc.finalize()
    return nc


_last_results = None


def kernel(x_1: np.ndarray, x_2: np.ndarray) -> np.ndarray:
    global _last_results
    import os
    import ml_dtypes
    from concourse.bass_utils import run_bass_kernel_spmd

    if "nc" not in _cache:
        _cache["nc"] = _build_nc()
    nc = _cache["nc"]

    bf = ml_dtypes.bfloat16
    x1b = np.asarray(x_1, dtype=np.float32).astype(bf)
    x2b = np.asarray(x_2, dtype=np.float32).astype(bf)

    in_maps = []
    for i in range(8):
        b, hh = i // 2, i % 2
        sl = slice(hh * HC, (hh + 1) * HC)
        in_maps.append({
            "x1s": np.ascontiguousarray(x1b[b, :, sl, :]),
            "x2s": np.ascontiguousarray(x2b[b, :, sl, :]),
        })
    kw = {}
    if os.environ.get("BASS_TRACE"):
        kw["tmpdir"] = os.environ.get("BASS_TRACE_DIR") or None
    res = run_bass_kernel_spmd(nc, in_maps, core_ids=list(range(8)), **kw)
    _last_results = res
    out = np.empty((B, D, H, W), dtype=np.float32)
    for i in range(8):
        b, hh = i // 2, i % 2
        out[b, :, hh * HC:(hh + 1) * HC, :] = res.results[i]["outs"]
    return out
